# revision 31
# baseline (speedup 1.0000x reference)
"""Trainium2 Bass kernel for nn_AngleTripletGenerator (DimeNet-style triplet
generation), distributed over 8 NeuronCores.

Strategy: data-parallel over center nodes (6250/core, padded to 6400 = 2
supertiles of 128 partitions x 25 nodes).  The angle/distance/mask grids are
symmetric in (j, k), so the device computes only the packed half-grid
H[n, j, d] for d = 1..8 with k = (j + d) mod 16 -- half the compute of the
full 16x16 grid and no diagonal masking op (d >= 1 excludes j == k).  The
mod-16 wraparound is handled by extended per-edge tiles x/y/z/d2 of width
24 = 16+8, so every grid operand is a plain affine AP ([b,24] stride view
with both j and d at stride 1 on the k side).

Angle math (division-free; Arctan LUT input stays in [-1, 1]):
  u = tanh((ln(max(cn2,1e-37)) - ln((G+1e-10)^2)) / 4) = (y-|x|)/(y+|x|)
  theta = ((atan(u) - pi/4) * sign(G + 1e-30) + pi/2) * mask
The asymmetric clamps reproduce atan2(0,0) = 0 for zero-length edges
(neighbor == center).  Distances: supertile 0 uses exp(0.5*ln(dsq + 1e-3))
(ln/exp share the natural_log_exp table set, no extra switch); supertile 1
uses Sqrt(dsq + 1e-3) -- one 2.9us op plus a table load beats the 5.9us
lnD+Exp pair inside the end-of-kernel serial ACT window.  The 1e-3 bias only
perturbs degenerate duplicate-neighbor slots (reference quirk value 1.0 vs
our ~0.03, ~500 of 12.8M slots).  The module patches the table catalog so
Ln resolves to natural_log_exp_and_others and Tanh to sigmoid_and_others
(Arctan's set), minimizing table switches; ACT op order is software-
pipelined across the two supertiles.

Host side does layout-only work: the pos gather (indirect DMA can't do it
efficiently), padding/transposes, the half-grid -> full-grid scatter (a fixed
permutation; every scattered value is device-computed), and the id3 outputs,
which are pure broadcasts of edge_index / arange with zero arithmetic.

Outputs from device: packed od (fp16), oa (fp16), om (u8), each [6400*128]
per core; host scatters into the [N,16,16] full grids and upcasts.
"""

import sys

sys.path.insert(0, "/opt/trn_rl_repo")

import numpy as np

import concourse.bass as bass
import concourse.bacc as bacc
import concourse.mybir as mybir
import concourse.tile as tile_mod
import concourse.hw_specs as _hw_specs


def _tables_ln_exp(arch):
    """Activation-table catalog tweaks (set order = set ids preserved):
    - hide the plain natural_log set so Ln is served from
      natural_log_exp_and_others and Exp needs no extra table switch;
    - hide tanh inside exp_and_others so Tanh resolves to sigmoid_and_others,
      the set Arctan needs anyway -- one load for the tanh+arctan block
      instead of two."""
    t = dict(_hw_specs.get_activation_tables(arch))
    if "natural_log" in t and "natural_log_exp_and_others" in t:
        t["natural_log"] = set()
    tanh = [f for f in t.get("exp_and_others", set()) if f.name == "Tanh"]
    if tanh and "sigmoid_and_others" in t:
        t["exp_and_others"] = t["exp_and_others"] - set(tanh)
    return t


bacc.get_activation_tables = _tables_ln_exp

F32 = mybir.dt.float32
FP16 = mybir.dt.float16
U8 = mybir.dt.uint8

N_NODES = 50000
DEG = 16
ND = 8               # half-grid depth: d = 1..8, k = (j+d) mod 16
GW = DEG * ND        # 128 grid elems per node
EXT = DEG + ND       # 24: extended edge tiles for the mod-16 wrap
N_CORES = 8
NPC = N_NODES // N_CORES   # 6250
P = 128
B = 25               # nodes per partition per supertile
NT = 2
ST = P * B           # 3200 nodes per supertile
NPC_PAD = NT * ST    # 6400
CUTOFF2 = 25.0
PI = float(np.pi)

A = mybir.AluOpType


def _ap(tile, offset, dims):
    """Free-dim AP on an SBUF tile: dims = [[stride, size], ...] (elements)."""
    base = tile[:]
    return bass.AP(base.tensor, base.offset + offset, [list(base.ap[0])] + dims)


def build_nc():
    nc = bacc.Bacc(None, target_bir_lowering=False, debug=False)

    # host layout: gpos row (t*128+p) = [3, B, 16] f32; cpos row = [3, B]
    gpos = nc.dram_tensor("gpos", [NT * P, 3 * B * DEG], F32, kind="ExternalInput")
    cpos = nc.dram_tensor("cpos", [NT * P, 3 * B], F32, kind="ExternalInput")
    phd = nc.dram_tensor("phd", [NT * P, B * GW], FP16, kind="ExternalOutput")
    pha = nc.dram_tensor("pha", [NT * P, B * GW], FP16, kind="ExternalOutput")
    phm = nc.dram_tensor("phm", [NT * P, B * GW], U8, kind="ExternalOutput")

    gpos_v = gpos[:].rearrange("(t p) f -> t p f", t=NT)
    gpos_cv = gpos[:].rearrange("(t p) (c f) -> t c p f", t=NT, c=3)
    cpos_v = cpos[:].rearrange("(t p) f -> t p f", t=NT)
    phd_v = phd[:].rearrange("(t p) f -> t p f", t=NT)
    pha_v = pha[:].rearrange("(t p) f -> t p f", t=NT)
    phd_hv = phd[:].rearrange("(t p) (h f) -> t h p f", t=NT, h=2)
    pha_hv = pha[:].rearrange("(t p) (h f) -> t h p f", t=NT, h=2)
    phm_v = phm[:].rearrange("(t p) f -> t p f", t=NT)

    TT = nc.vector.tensor_tensor
    TS = nc.vector.tensor_scalar
    STT = nc.vector.scalar_tensor_tensor
    ACT = nc.scalar.activation
    AF = mybir.ActivationFunctionType

    with tile_mod.TileContext(nc) as tc:
        with tc.tile_pool(name="const", bufs=1) as cpool, tc.tile_pool(
            name="work", bufs=2
        ) as pool:
            b_zero = cpool.tile([P, 1], F32, tag="b_zero")
            nc.vector.memset(b_zero[:], 0.0)
            b_sq = cpool.tile([P, 1], F32, tag="b_sq")
            nc.vector.memset(b_sq[:], 1e-10)
            b_sgn = cpool.tile([P, 1], F32, tag="b_sgn")
            nc.vector.memset(b_sgn[:], 1e-30)
            b_lnd = cpool.tile([P, 1], F32, tag="b_lnd")
            nc.vector.memset(b_lnd[:], 1e-3)
            b_abs = cpool.tile([P, 1], F32, tag="b_abs")
            nc.vector.memset(b_abs[:], 1e-37)
            BZ = b_zero[:, :1]
            tiles = {}

            def fr1(t):
                """Loads, R1 extraction, d2, validity; POOL A and S."""
                eng = nc.gpsimd
                cpt = pool.tile([P, 3 * B], F32, tag="cpt")
                eng.dma_start(out=cpt[:], in_=cpos_v[t])
                gath = pool.tile([P, 3 * B * DEG], F32, tag="gath")
                if t == 0:
                    w = B * DEG
                    for ci in range(3):
                        eng.dma_start(out=gath[:, ci * w:(ci + 1) * w],
                                      in_=gpos_cv[t][ci])
                else:
                    eng.dma_start(out=gath[:], in_=gpos_v[t])

                xe = pool.tile([P, B * EXT], F32, tag="xe")
                ye = pool.tile([P, B * EXT], F32, tag="ye")
                ze = pool.tile([P, B * EXT], F32, tag="ze")
                d2e = pool.tile([P, B * EXT], F32, tag="d2e")
                ve = pool.tile([P, B * EXT], F32, tag="ve")
                tmp = pool.tile([P, B * DEG], F32, tag="tmp")

                for ci, dst in enumerate((xe, ye, ze)):
                    TT(
                        out=_ap(dst, 0, [[EXT, B], [1, DEG]]),
                        in0=_ap(gath, ci * B * DEG, [[DEG, B], [1, DEG]]),
                        in1=_ap(cpt, ci * B, [[1, B], [0, DEG]]),
                        op=A.subtract,
                    )
                # d2 = x^2 + y^2 + z^2 (squares on ACT, adds on DVE)
                d2m = _ap(d2e, 0, [[EXT, B], [1, DEG]])
                tm = _ap(tmp, 0, [[DEG, B], [1, DEG]])
                xm = _ap(xe, 0, [[EXT, B], [1, DEG]])
                ym = _ap(ye, 0, [[EXT, B], [1, DEG]])
                zm = _ap(ze, 0, [[EXT, B], [1, DEG]])
                ACT(out=d2m, in_=xm, func=AF.Square, bias=BZ)
                ACT(out=tm, in_=ym, func=AF.Square, bias=BZ)
                TT(out=d2m, in0=d2m, in1=tm, op=A.add)
                ACT(out=tm, in_=zm, func=AF.Square, bias=BZ)
                TT(out=d2m, in0=d2m, in1=tm, op=A.add)
                # wrap copies: ext[16:24] = main[0:8] (ACT Copy, filler func)
                for srct in (xe, ye, ze, d2e):
                    ACT(out=_ap(srct, DEG, [[EXT, B], [1, ND]]),
                        in_=_ap(srct, 0, [[EXT, B], [1, ND]]), func=AF.Copy)
                TS(out=ve[:], in0=d2e[:], scalar1=CUTOFF2, scalar2=None,
                   op0=A.is_le)

                G = pool.tile([P, B * GW], F32, tag="G")
                T1 = pool.tile([P, B * GW], F32, tag="T1")
                T2 = pool.tile([P, B * GW], F32, tag="T2")
                T3 = pool.tile([P, B * GW], F32, tag="T3")
                M16 = pool.tile([P, B * GW], FP16, tag="M16")
                LNB = pool.tile([P, B * GW], FP16, tag="LNB")
                SG = pool.tile([P, B * GW], FP16, tag="SG")
                F1 = pool.tile([P, B * GW], FP16, tag="F1")
                F2 = pool.tile([P, B * GW], FP16, tag="F2")

                def jside(tl):
                    return _ap(tl, 0, [[EXT, B], [1, DEG], [0, ND]])

                def kside(tl):
                    return _ap(tl, 1, [[EXT, B], [1, DEG], [1, ND]])

                def gv(tl):
                    return _ap(tl, 0, [[GW, B], [ND, DEG], [1, ND]])

                # A = d2j*d2k and S = d2j+d2k (DVE: GpSimd elementwise
                # contends with DVE for SBUF and slows both engines)
                TT(out=gv(T2), in0=jside(d2e), in1=kside(d2e), op=A.mult)
                TT(out=gv(T3), in0=jside(d2e), in1=kside(d2e), op=A.add)
                tiles[t] = dict(G=G, T1=T1, T2=T2, T3=T3, M16=M16, SG=SG,
                                F1=F1, F2=F2, LNB=LNB, xe=xe, ye=ye, ze=ze,
                                ve=ve, jside=jside, kside=kside, gv=gv)

            def fr2(t):
                """Mask (DVE f32 -> T1, ACT-converted to fp16), G, G2, Sign."""
                d = tiles[t]
                jside, kside, gv = d["jside"], d["kside"], d["gv"]
                G, T1 = d["G"], d["T1"]
                TT(out=gv(G), in0=jside(d["xe"]), in1=kside(d["xe"]),
                   op=A.mult)
                TT(out=gv(T1), in0=jside(d["ye"]), in1=kside(d["ye"]),
                   op=A.mult)
                TT(out=G[:], in0=G[:], in1=T1[:], op=A.add)
                TT(out=gv(T1), in0=jside(d["ze"]), in1=kside(d["ze"]),
                   op=A.mult)
                TT(out=G[:], in0=G[:], in1=T1[:], op=A.add)
                ACT(out=T1[:], in_=G[:], func=AF.Square, bias=b_sq[:, :1])


            def sgst(t):
                # Sign only feeds the tail's w-multiply; emitted late so it
                # stays off the G2 -> ln -> tanh critical chain
                d = tiles[t]
                ACT(out=d["SG"][:], in_=d["G"][:], func=AF.Sign,
                    bias=b_sgn[:, :1])

            def dsqst(t):
                d = tiles[t]
                STT(out=d["T3"][:], in0=d["G"][:], scalar=-2.0, in1=d["T3"][:],
                    op0=A.mult, op1=A.add)

            def maskst(t):
                d = tiles[t]
                jside, kside, gv = d["jside"], d["kside"], d["gv"]
                TT(out=gv(d["M16"]), in0=jside(d["ve"]), in1=kside(d["ve"]),
                   op=A.mult)
                nc.gpsimd.dma_start(out=phm_v[t], in_=d["M16"][:])  # fp16->u8

            def frontB(t):
                d = tiles[t]
                TT(out=d["T2"][:], in0=d["T2"][:], in1=d["T1"][:],
                   op=A.subtract)
                TS(out=d["T2"][:], in0=d["T2"][:], scalar1=1e-37, scalar2=None,
                   op0=A.max)

            def mid(t):
                """natural_log set: lnB, and for st0 lnD (st1 computes dist
                via Sqrt instead -- one 2.9us op + a table load beats the
                5.9us lnD+Exp pair inside the end-of-kernel ACT window).
                lnA/lnB outputs are fp16 so the t-subtraction runs 2x."""
                d = tiles[t]
                ACT(out=d["LNB"][:], in_=d["T1"][:], func=AF.Ln, bias=BZ)
                if t == 0:
                    ACT(out=d["T3"][:], in_=d["T3"][:], func=AF.Ln,
                        bias=b_lnd[:, :1])

            def ln_a(t):
                d = tiles[t]
                ACT(out=d["F1"][:], in_=d["T2"][:], func=AF.Ln, bias=BZ)

            def dist_exp(t):
                d = tiles[t]
                if t == 0:
                    ACT(out=d["F2"][:], in_=d["T3"][:], func=AF.Exp, bias=BZ,
                        scale=0.5)
                else:
                    ACT(out=d["F2"][:], in_=d["T3"][:], func=AF.Sqrt,
                        bias=b_lnd[:, :1])

            def tsub(t):
                d = tiles[t]
                TT(out=d["F1"][:], in0=d["F1"][:], in1=d["LNB"][:],
                   op=A.subtract)

            def back(t):
                d = tiles[t]
                ACT(out=d["F1"][:], in_=d["F1"][:], func=AF.Tanh, bias=BZ,
                    scale=0.25)
                ACT(out=d["F1"][:], in_=d["F1"][:], func=AF.Arctan, bias=BZ)

            def tail(t):
                d = tiles[t]
                TT(out=d["F2"][:], in0=d["F2"][:], in1=d["M16"][:], op=A.mult)
                nc.sync.dma_start(out=phd_v[t], in_=d["F2"][:])
                TS(out=d["F1"][:], in0=d["F1"][:], scalar1=-PI / 4,
                   scalar2=None, op0=A.add)
                TT(out=d["F1"][:], in0=d["F1"][:], in1=d["SG"][:], op=A.mult)
                TS(out=d["F1"][:], in0=d["F1"][:], scalar1=PI / 2,
                   scalar2=None, op0=A.add)
                TT(out=d["F1"][:], in0=d["F1"][:], in1=d["M16"][:], op=A.mult)
                nc.sync.dma_start(out=pha_v[t], in_=d["F1"][:])

            HW = B * GW // 2

            def hs(tl, h):
                return tl[:, h * HW:(h + 1) * HW]

            def ln_a_h(t, h):
                d = tiles[t]
                ACT(out=hs(d["F1"], h), in_=hs(d["T2"], h), func=AF.Ln,
                    bias=BZ)

            def tsub_h(t, h):
                d = tiles[t]
                TT(out=hs(d["F1"], h), in0=hs(d["F1"], h),
                   in1=hs(d["LNB"], h), op=A.subtract)

            def back_h(t, h):
                d = tiles[t]
                ACT(out=hs(d["F1"], h), in_=hs(d["F1"], h), func=AF.Tanh,
                    bias=BZ, scale=0.25)
                ACT(out=hs(d["F1"], h), in_=hs(d["F1"], h), func=AF.Arctan,
                    bias=BZ)

            def tail_h(t, h):
                d = tiles[t]
                TT(out=hs(d["F2"], h), in0=hs(d["F2"], h),
                   in1=hs(d["M16"], h), op=A.mult)
                nc.sync.dma_start(out=phd_hv[t][h], in_=hs(d["F2"], h))
                TS(out=hs(d["F1"], h), in0=hs(d["F1"], h), scalar1=-PI / 4,
                   scalar2=None, op0=A.add)
                TT(out=hs(d["F1"], h), in0=hs(d["F1"], h),
                   in1=hs(d["SG"], h), op=A.mult)
                TS(out=hs(d["F1"], h), in0=hs(d["F1"], h), scalar1=PI / 2,
                   scalar2=None, op0=A.add)
                TT(out=hs(d["F1"], h), in0=hs(d["F1"], h),
                   in1=hs(d["M16"], h), op=A.mult)
                nc.sync.dma_start(out=pha_hv[t][h], in_=hs(d["F1"], h))

            # pipeline: ln-set work grouped, exp joins it (patched catalog);
            # st1's serial angle endgame is half-split to pipeline ACT/DVE.
            fr1(0)
            fr2(0)
            dsqst(0)
            maskst(0)
            fr1(1)
            frontB(0)
            mid(0)
            dist_exp(0)
            ln_a(0)
            sgst(0)
            fr2(1)
            tsub(0)
            dsqst(1)
            maskst(1)
            frontB(1)
            mid(1)
            sgst(1)
            dist_exp(1)
            back(0)
            tail(0)
            ln_a_h(1, 0)
            tsub_h(1, 0)
            ln_a_h(1, 1)
            back_h(1, 0)
            tsub_h(1, 1)
            back_h(1, 1)
            tail_h(1, 0)
            tail_h(1, 1)

    return nc


_NC_CACHE = {}


def _get_nc():
    if "nc" not in _NC_CACHE:
        nc = build_nc()
        nc.finalize()
        _NC_CACHE["nc"] = nc
    return _NC_CACHE["nc"]


# half-grid -> full-grid scatter indices (fixed permutation)
_JF = np.broadcast_to(np.arange(DEG, dtype=np.int64)[:, None], (DEG, ND))
_KF = (np.arange(DEG, dtype=np.int64)[:, None]
       + np.arange(1, ND + 1, dtype=np.int64)[None, :]) % DEG

_OI_CACHE = {}


def _shard_inputs(pos, col2d):
    in_maps = []
    for c in range(N_CORES):
        lo = c * NPC
        colc = col2d[lo:lo + NPC]
        gp = np.zeros((NPC_PAD, DEG, 3), dtype=np.float32)
        gp[:NPC] = pos[colc]
        # -> [NT, P, 3, B, 16] -> [NT*P, 3*B*16]
        gp = gp.reshape(NT, P, B, DEG, 3).transpose(0, 1, 4, 2, 3)
        gp = np.ascontiguousarray(gp).reshape(NT * P, 3 * B * DEG)
        cp = np.zeros((NPC_PAD, 3), dtype=np.float32)
        cp[:NPC] = pos[lo:lo + NPC]
        cp = cp.reshape(NT, P, B, 3).transpose(0, 1, 3, 2)
        cp = np.ascontiguousarray(cp).reshape(NT * P, 3 * B)
        in_maps.append({"gpos": gp, "cpos": cp})
    return in_maps


def kernel(pos, edge_index, _trace=False):
    """Full-input / full-output entry point. Returns the same tuple as
    reference(): (id3_i, id3_j, id3_k, distances_jk, angles, mask)."""
    from concourse.bass_utils import run_bass_kernel_spmd

    pos = np.asarray(pos, dtype=np.float32)
    edge_index = np.asarray(edge_index, dtype=np.int32)
    n = pos.shape[0]
    deg = edge_index.shape[1] // n
    assert n == N_NODES and deg == DEG

    col2d = edge_index[1].reshape(n, deg)

    nc = _get_nc()
    in_maps = _shard_inputs(pos, col2d)
    res = run_bass_kernel_spmd(
        nc, in_maps, core_ids=list(range(N_CORES)), trace=_trace
    )

    od = np.zeros((n, DEG, DEG), dtype=np.float32)
    oa = np.zeros((n, DEG, DEG), dtype=np.float32)
    om = np.zeros((n, DEG, DEG), dtype=bool)
    for c in range(N_CORES):
        lo = c * NPC
        r = res.results[c]
        hd = np.asarray(r["phd"]).reshape(NPC_PAD, DEG, ND)[:NPC]
        ha = np.asarray(r["pha"]).reshape(NPC_PAD, DEG, ND)[:NPC]
        hm = np.asarray(r["phm"]).reshape(NPC_PAD, DEG, ND)[:NPC] != 0
        sl = slice(lo, lo + NPC)
        od[sl][:, _JF, _KF] = hd
        od[sl][:, _KF, _JF] = hd
        oa[sl][:, _JF, _KF] = ha
        oa[sl][:, _KF, _JF] = ha
        om[sl][:, _JF, _KF] = hm
        om[sl][:, _KF, _JF] = hm

    if "oi" not in _OI_CACHE:
        _OI_CACHE["oi"] = np.repeat(
            np.arange(n, dtype=np.int32), DEG * DEG
        )
    oi = _OI_CACHE["oi"]
    oj = np.ascontiguousarray(
        np.broadcast_to(col2d[:, :, None], (n, DEG, DEG))
    ).reshape(-1)
    ok = np.ascontiguousarray(
        np.broadcast_to(col2d[:, None, :], (n, DEG, DEG))
    ).reshape(-1)

    ret = (oi, oj, ok, od.reshape(-1), oa.reshape(-1), om.reshape(-1))
    if _trace:
        return ret, res
    return ret


# revision 32
# speedup vs baseline: 1.1896x; 1.1896x over previous
"""Trainium2 Bass kernel for nn_AngleTripletGenerator (DimeNet-style triplet
generation), distributed over 8 NeuronCores.

Strategy: data-parallel over center nodes (6250/core, padded to 6400 = 2
supertiles of 128 partitions x 25 nodes).  The angle/distance/mask grids are
symmetric in (j, k), so the device computes only the packed half-grid
H[n, j, d] for d = 1..8 with k = (j + d) mod 16 -- half the compute of the
full 16x16 grid and no diagonal masking op (d >= 1 excludes j == k).  The
mod-16 wraparound is handled by extended per-edge tiles x/y/z/d2 of width
24 = 16+8, so every grid operand is a plain affine AP ([b,24] stride view
with both j and d at stride 1 on the k side).

Angle math (division-free; Arctan LUT input stays in [-1, 1]):
  u = tanh((ln(max(cn2,1e-37)) - ln((G+1e-10)^2)) / 4) = (y-|x|)/(y+|x|)
  theta = ((atan(u) - pi/4) * sign(G + 1e-30) + pi/2) * mask
The asymmetric clamps reproduce atan2(0,0) = 0 for zero-length edges
(neighbor == center).  Distances: supertile 0 uses exp(0.5*ln(dsq + 1e-3))
(ln/exp share the natural_log_exp table set, no extra switch); supertile 1
uses Sqrt(dsq + 1e-3) -- one 2.9us op plus a table load beats the 5.9us
lnD+Exp pair inside the end-of-kernel serial ACT window.  The 1e-3 bias only
perturbs degenerate duplicate-neighbor slots (reference quirk value 1.0 vs
our ~0.03, ~500 of 12.8M slots).  The module patches the table catalog so
Ln resolves to natural_log_exp_and_others and Tanh to sigmoid_and_others
(Arctan's set), minimizing table switches; ACT op order is software-
pipelined across the two supertiles.

Host side does layout-only work: the pos gather (indirect DMA can't do it
efficiently), padding/transposes, the half-grid -> full-grid scatter (a fixed
permutation; every scattered value is device-computed), and the id3 outputs,
which are pure broadcasts of edge_index / arange with zero arithmetic.

Outputs from device: packed od (fp16), oa (fp16), om (u8), each [6400*128]
per core; host scatters into the [N,16,16] full grids and upcasts.
"""

import sys

sys.path.insert(0, "/opt/trn_rl_repo")

import numpy as np

import concourse.bass as bass
import concourse.bacc as bacc
import concourse.mybir as mybir
import concourse.tile as tile_mod
import concourse.hw_specs as _hw_specs


def _tables_ln_exp(arch):
    """Activation-table catalog tweaks (set order = set ids preserved):
    - hide the plain natural_log set so Ln is served from
      natural_log_exp_and_others and Exp needs no extra table switch;
    - hide tanh inside exp_and_others so Tanh resolves to sigmoid_and_others,
      the set Arctan needs anyway -- one load for the tanh+arctan block
      instead of two."""
    t = dict(_hw_specs.get_activation_tables(arch))
    if "natural_log" in t and "natural_log_exp_and_others" in t:
        t["natural_log"] = set()
    tanh = [f for f in t.get("exp_and_others", set()) if f.name == "Tanh"]
    if tanh and "sigmoid_and_others" in t:
        t["exp_and_others"] = t["exp_and_others"] - set(tanh)
    return t


bacc.get_activation_tables = _tables_ln_exp

F32 = mybir.dt.float32
FP16 = mybir.dt.float16
U8 = mybir.dt.uint8

N_NODES = 50000
DEG = 16
ND = 8               # half-grid depth: d = 1..8, k = (j+d) mod 16
GW = DEG * ND        # 128 grid elems per node
EXT = DEG + ND       # 24: extended edge tiles for the mod-16 wrap
N_CORES = 8
NPC = N_NODES // N_CORES   # 6250
P = 128
B = 25               # nodes per partition per supertile
NT = 2
ST = P * B           # 3200 nodes per supertile
NPC_PAD = NT * ST    # 6400
CUTOFF2 = 25.0
PI = float(np.pi)

A = mybir.AluOpType


def _ap(tile, offset, dims):
    """Free-dim AP on an SBUF tile: dims = [[stride, size], ...] (elements)."""
    base = tile[:]
    return bass.AP(base.tensor, base.offset + offset, [list(base.ap[0])] + dims)


def build_nc():
    nc = bacc.Bacc(None, target_bir_lowering=False, debug=False)

    # host layout: gpos row (t*128+p) = [3, B, 16] f32; cpos row = [3, B]
    gpos = nc.dram_tensor("gpos", [NT * P, 3 * B * DEG], F32, kind="ExternalInput")
    cpos = nc.dram_tensor("cpos", [NT * P, 3 * B], F32, kind="ExternalInput")
    phd = nc.dram_tensor("phd", [NT * P, B * GW], FP16, kind="ExternalOutput")
    pha = nc.dram_tensor("pha", [NT * P, B * GW], FP16, kind="ExternalOutput")
    phm = nc.dram_tensor("phm", [NT * P, B * GW], U8, kind="ExternalOutput")

    gpos_v = gpos[:].rearrange("(t p) f -> t p f", t=NT)
    gpos_cv = gpos[:].rearrange("(t p) (c f) -> t c p f", t=NT, c=3)
    cpos_v = cpos[:].rearrange("(t p) f -> t p f", t=NT)
    phd_v = phd[:].rearrange("(t p) f -> t p f", t=NT)
    pha_v = pha[:].rearrange("(t p) f -> t p f", t=NT)
    phd_hv = phd[:].rearrange("(t p) (h f) -> t h p f", t=NT, h=2)
    pha_hv = pha[:].rearrange("(t p) (h f) -> t h p f", t=NT, h=2)
    phm_v = phm[:].rearrange("(t p) f -> t p f", t=NT)

    TT = nc.vector.tensor_tensor
    TS = nc.vector.tensor_scalar
    STT = nc.vector.scalar_tensor_tensor
    ACT = nc.scalar.activation
    AF = mybir.ActivationFunctionType

    with tile_mod.TileContext(nc) as tc:
        with tc.tile_pool(name="const", bufs=1) as cpool, tc.tile_pool(
            name="work", bufs=2
        ) as pool:
            b_zero = cpool.tile([P, 1], F32, tag="b_zero")
            nc.vector.memset(b_zero[:], 0.0)
            b_sq = cpool.tile([P, 1], F32, tag="b_sq")
            nc.vector.memset(b_sq[:], 1e-10)
            b_sgn = cpool.tile([P, 1], F32, tag="b_sgn")
            nc.vector.memset(b_sgn[:], 1e-30)
            b_lnd = cpool.tile([P, 1], F32, tag="b_lnd")
            nc.vector.memset(b_lnd[:], 1e-3)
            b_abs = cpool.tile([P, 1], F32, tag="b_abs")
            nc.vector.memset(b_abs[:], 1e-37)
            BZ = b_zero[:, :1]
            tiles = {}

            def fr1(t):
                """Loads, R1 extraction, d2, validity; POOL A and S."""
                eng = nc.gpsimd
                cpt = pool.tile([P, 3 * B], F32, tag="cpt")
                eng.dma_start(out=cpt[:], in_=cpos_v[t])
                gath = pool.tile([P, 3 * B * DEG], F32, tag="gath")
                if t == 0:
                    w = B * DEG
                    for ci in range(3):
                        eng.dma_start(out=gath[:, ci * w:(ci + 1) * w],
                                      in_=gpos_cv[t][ci])
                else:
                    eng.dma_start(out=gath[:], in_=gpos_v[t])

                xe = pool.tile([P, B * EXT], F32, tag="xe")
                ye = pool.tile([P, B * EXT], F32, tag="ye")
                ze = pool.tile([P, B * EXT], F32, tag="ze")
                d2e = pool.tile([P, B * EXT], F32, tag="d2e")
                ve = pool.tile([P, B * EXT], F32, tag="ve")
                tmp = pool.tile([P, B * DEG], F32, tag="tmp")

                for ci, dst in enumerate((xe, ye, ze)):
                    TT(
                        out=_ap(dst, 0, [[EXT, B], [1, DEG]]),
                        in0=_ap(gath, ci * B * DEG, [[DEG, B], [1, DEG]]),
                        in1=_ap(cpt, ci * B, [[1, B], [0, DEG]]),
                        op=A.subtract,
                    )
                # d2 = x^2 + y^2 + z^2 (squares on ACT, adds on DVE)
                d2m = _ap(d2e, 0, [[EXT, B], [1, DEG]])
                tm = _ap(tmp, 0, [[DEG, B], [1, DEG]])
                xm = _ap(xe, 0, [[EXT, B], [1, DEG]])
                ym = _ap(ye, 0, [[EXT, B], [1, DEG]])
                zm = _ap(ze, 0, [[EXT, B], [1, DEG]])
                ACT(out=d2m, in_=xm, func=AF.Square, bias=BZ)
                ACT(out=tm, in_=ym, func=AF.Square, bias=BZ)
                TT(out=d2m, in0=d2m, in1=tm, op=A.add)
                ACT(out=tm, in_=zm, func=AF.Square, bias=BZ)
                TT(out=d2m, in0=d2m, in1=tm, op=A.add)
                # wrap copies: ext[16:24] = main[0:8] (ACT Copy, filler func)
                for srct in (xe, ye, ze, d2e):
                    ACT(out=_ap(srct, DEG, [[EXT, B], [1, ND]]),
                        in_=_ap(srct, 0, [[EXT, B], [1, ND]]), func=AF.Copy)
                TS(out=ve[:], in0=d2e[:], scalar1=CUTOFF2, scalar2=None,
                   op0=A.is_le)

                G = pool.tile([P, B * GW], F32, tag="G")
                T1 = pool.tile([P, B * GW], F32, tag="T1")
                T2 = pool.tile([P, B * GW], F32, tag="T2")
                T3 = pool.tile([P, B * GW], F32, tag="T3")
                M16 = pool.tile([P, B * GW], FP16, tag="M16")
                LNB = pool.tile([P, B * GW], FP16, tag="LNB")
                SG = pool.tile([P, B * GW], FP16, tag="SG")
                F1 = pool.tile([P, B * GW], FP16, tag="F1")
                F2 = pool.tile([P, B * GW], FP16, tag="F2")

                def jside(tl):
                    return _ap(tl, 0, [[EXT, B], [1, DEG], [0, ND]])

                def kside(tl):
                    return _ap(tl, 1, [[EXT, B], [1, DEG], [1, ND]])

                def gv(tl):
                    return _ap(tl, 0, [[GW, B], [ND, DEG], [1, ND]])

                # A = d2j*d2k and S = d2j+d2k (DVE: GpSimd elementwise
                # contends with DVE for SBUF and slows both engines)
                TT(out=gv(T2), in0=jside(d2e), in1=kside(d2e), op=A.mult)
                TT(out=gv(T3), in0=jside(d2e), in1=kside(d2e), op=A.add)
                tiles[t] = dict(G=G, T1=T1, T2=T2, T3=T3, M16=M16, SG=SG,
                                F1=F1, F2=F2, LNB=LNB, xe=xe, ye=ye, ze=ze,
                                ve=ve, jside=jside, kside=kside, gv=gv)

            def fr2(t):
                """Mask (DVE f32 -> T1, ACT-converted to fp16), G, G2, Sign."""
                d = tiles[t]
                jside, kside, gv = d["jside"], d["kside"], d["gv"]
                G, T1 = d["G"], d["T1"]
                TT(out=gv(G), in0=jside(d["xe"]), in1=kside(d["xe"]),
                   op=A.mult)
                TT(out=gv(T1), in0=jside(d["ye"]), in1=kside(d["ye"]),
                   op=A.mult)
                TT(out=G[:], in0=G[:], in1=T1[:], op=A.add)
                TT(out=gv(T1), in0=jside(d["ze"]), in1=kside(d["ze"]),
                   op=A.mult)
                TT(out=G[:], in0=G[:], in1=T1[:], op=A.add)
                ACT(out=T1[:], in_=G[:], func=AF.Square, bias=b_sq[:, :1])


            def sgst(t):
                # Sign only feeds the tail's w-multiply; emitted late so it
                # stays off the G2 -> ln -> tanh critical chain
                d = tiles[t]
                ACT(out=d["SG"][:], in_=d["G"][:], func=AF.Sign,
                    bias=b_sgn[:, :1])

            def dsqst(t):
                d = tiles[t]
                STT(out=d["T3"][:], in0=d["G"][:], scalar=-2.0, in1=d["T3"][:],
                    op0=A.mult, op1=A.add)

            def maskst(t):
                d = tiles[t]
                jside, kside, gv = d["jside"], d["kside"], d["gv"]
                TT(out=gv(d["M16"]), in0=jside(d["ve"]), in1=kside(d["ve"]),
                   op=A.mult)
                nc.gpsimd.dma_start(out=phm_v[t], in_=d["M16"][:])  # fp16->u8

            def frontB(t):
                d = tiles[t]
                TT(out=d["T2"][:], in0=d["T2"][:], in1=d["T1"][:],
                   op=A.subtract)
                TS(out=d["T2"][:], in0=d["T2"][:], scalar1=1e-37, scalar2=None,
                   op0=A.max)

            def mid(t):
                """natural_log set: lnB, and for st0 lnD (st1 computes dist
                via Sqrt instead -- one 2.9us op + a table load beats the
                5.9us lnD+Exp pair inside the end-of-kernel ACT window).
                lnA/lnB outputs are fp16 so the t-subtraction runs 2x."""
                d = tiles[t]
                ACT(out=d["LNB"][:], in_=d["T1"][:], func=AF.Ln, bias=BZ)
                if t == 0:
                    ACT(out=d["T3"][:], in_=d["T3"][:], func=AF.Ln,
                        bias=b_lnd[:, :1])

            def ln_a(t):
                d = tiles[t]
                ACT(out=d["F1"][:], in_=d["T2"][:], func=AF.Ln, bias=BZ)

            def dist_exp(t):
                d = tiles[t]
                if t == 0:
                    ACT(out=d["F2"][:], in_=d["T3"][:], func=AF.Exp, bias=BZ,
                        scale=0.5)
                else:
                    ACT(out=d["F2"][:], in_=d["T3"][:], func=AF.Sqrt,
                        bias=b_lnd[:, :1])

            def tsub(t):
                d = tiles[t]
                TT(out=d["F1"][:], in0=d["F1"][:], in1=d["LNB"][:],
                   op=A.subtract)

            def back(t):
                d = tiles[t]
                ACT(out=d["F1"][:], in_=d["F1"][:], func=AF.Tanh, bias=BZ,
                    scale=0.25)
                ACT(out=d["F1"][:], in_=d["F1"][:], func=AF.Arctan, bias=BZ)

            def tail(t):
                d = tiles[t]
                TT(out=d["F2"][:], in0=d["F2"][:], in1=d["M16"][:], op=A.mult)
                nc.sync.dma_start(out=phd_v[t], in_=d["F2"][:])
                TS(out=d["F1"][:], in0=d["F1"][:], scalar1=-PI / 4,
                   scalar2=None, op0=A.add)
                TT(out=d["F1"][:], in0=d["F1"][:], in1=d["SG"][:], op=A.mult)
                TS(out=d["F1"][:], in0=d["F1"][:], scalar1=PI / 2,
                   scalar2=None, op0=A.add)
                TT(out=d["F1"][:], in0=d["F1"][:], in1=d["M16"][:], op=A.mult)
                nc.sync.dma_start(out=pha_v[t], in_=d["F1"][:])

            HW = B * GW // 2

            def hs(tl, h):
                return tl[:, h * HW:(h + 1) * HW]

            def ln_a_h(t, h):
                d = tiles[t]
                ACT(out=hs(d["F1"], h), in_=hs(d["T2"], h), func=AF.Ln,
                    bias=BZ)

            def tsub_h(t, h):
                d = tiles[t]
                TT(out=hs(d["F1"], h), in0=hs(d["F1"], h),
                   in1=hs(d["LNB"], h), op=A.subtract)

            def back_h(t, h):
                d = tiles[t]
                ACT(out=hs(d["F1"], h), in_=hs(d["F1"], h), func=AF.Tanh,
                    bias=BZ, scale=0.25)
                ACT(out=hs(d["F1"], h), in_=hs(d["F1"], h), func=AF.Arctan,
                    bias=BZ)

            def tail_h(t, h):
                d = tiles[t]
                TT(out=hs(d["F2"], h), in0=hs(d["F2"], h),
                   in1=hs(d["M16"], h), op=A.mult)
                nc.sync.dma_start(out=phd_hv[t][h], in_=hs(d["F2"], h))
                TS(out=hs(d["F1"], h), in0=hs(d["F1"], h), scalar1=-PI / 4,
                   scalar2=None, op0=A.add)
                TT(out=hs(d["F1"], h), in0=hs(d["F1"], h),
                   in1=hs(d["SG"], h), op=A.mult)
                TS(out=hs(d["F1"], h), in0=hs(d["F1"], h), scalar1=PI / 2,
                   scalar2=None, op0=A.add)
                TT(out=hs(d["F1"], h), in0=hs(d["F1"], h),
                   in1=hs(d["M16"], h), op=A.mult)
                nc.sync.dma_start(out=pha_hv[t][h], in_=hs(d["F1"], h))

            # pipeline: ln-set work grouped, exp joins it (patched catalog);
            # st1's serial angle endgame is half-split to pipeline ACT/DVE.
            fr1(0)
            fr2(0)
            dsqst(0)
            maskst(0)
            fr1(1)
            frontB(0)
            mid(0)
            dist_exp(0)
            ln_a(0)
            sgst(0)
            fr2(1)
            tsub(0)
            dsqst(1)
            frontB(1)
            maskst(1)
            mid(1)
            sgst(1)
            dist_exp(1)
            back(0)
            tail(0)
            ln_a_h(1, 0)
            tsub_h(1, 0)
            ln_a_h(1, 1)
            back_h(1, 0)
            tsub_h(1, 1)
            back_h(1, 1)
            tail_h(1, 0)
            tail_h(1, 1)

    return nc


_NC_CACHE = {}


def _get_nc():
    if "nc" not in _NC_CACHE:
        nc = build_nc()
        nc.finalize()
        _NC_CACHE["nc"] = nc
    return _NC_CACHE["nc"]


# half-grid -> full-grid scatter indices (fixed permutation)
_JF = np.broadcast_to(np.arange(DEG, dtype=np.int64)[:, None], (DEG, ND))
_KF = (np.arange(DEG, dtype=np.int64)[:, None]
       + np.arange(1, ND + 1, dtype=np.int64)[None, :]) % DEG

_OI_CACHE = {}


def _shard_inputs(pos, col2d):
    in_maps = []
    for c in range(N_CORES):
        lo = c * NPC
        colc = col2d[lo:lo + NPC]
        gp = np.zeros((NPC_PAD, DEG, 3), dtype=np.float32)
        gp[:NPC] = pos[colc]
        # -> [NT, P, 3, B, 16] -> [NT*P, 3*B*16]
        gp = gp.reshape(NT, P, B, DEG, 3).transpose(0, 1, 4, 2, 3)
        gp = np.ascontiguousarray(gp).reshape(NT * P, 3 * B * DEG)
        cp = np.zeros((NPC_PAD, 3), dtype=np.float32)
        cp[:NPC] = pos[lo:lo + NPC]
        cp = cp.reshape(NT, P, B, 3).transpose(0, 1, 3, 2)
        cp = np.ascontiguousarray(cp).reshape(NT * P, 3 * B)
        in_maps.append({"gpos": gp, "cpos": cp})
    return in_maps


def kernel(pos, edge_index, _trace=False):
    """Full-input / full-output entry point. Returns the same tuple as
    reference(): (id3_i, id3_j, id3_k, distances_jk, angles, mask)."""
    from concourse.bass_utils import run_bass_kernel_spmd

    pos = np.asarray(pos, dtype=np.float32)
    edge_index = np.asarray(edge_index, dtype=np.int32)
    n = pos.shape[0]
    deg = edge_index.shape[1] // n
    assert n == N_NODES and deg == DEG

    col2d = edge_index[1].reshape(n, deg)

    nc = _get_nc()
    in_maps = _shard_inputs(pos, col2d)
    res = run_bass_kernel_spmd(
        nc, in_maps, core_ids=list(range(N_CORES)), trace=_trace
    )

    od = np.zeros((n, DEG, DEG), dtype=np.float32)
    oa = np.zeros((n, DEG, DEG), dtype=np.float32)
    om = np.zeros((n, DEG, DEG), dtype=bool)
    for c in range(N_CORES):
        lo = c * NPC
        r = res.results[c]
        hd = np.asarray(r["phd"]).reshape(NPC_PAD, DEG, ND)[:NPC]
        ha = np.asarray(r["pha"]).reshape(NPC_PAD, DEG, ND)[:NPC]
        hm = np.asarray(r["phm"]).reshape(NPC_PAD, DEG, ND)[:NPC] != 0
        sl = slice(lo, lo + NPC)
        od[sl][:, _JF, _KF] = hd
        od[sl][:, _KF, _JF] = hd
        oa[sl][:, _JF, _KF] = ha
        oa[sl][:, _KF, _JF] = ha
        om[sl][:, _JF, _KF] = hm
        om[sl][:, _KF, _JF] = hm

    if "oi" not in _OI_CACHE:
        _OI_CACHE["oi"] = np.repeat(
            np.arange(n, dtype=np.int32), DEG * DEG
        )
    oi = _OI_CACHE["oi"]
    oj = np.ascontiguousarray(
        np.broadcast_to(col2d[:, :, None], (n, DEG, DEG))
    ).reshape(-1)
    ok = np.ascontiguousarray(
        np.broadcast_to(col2d[:, None, :], (n, DEG, DEG))
    ).reshape(-1)

    ret = (oi, oj, ok, od.reshape(-1), oa.reshape(-1), om.reshape(-1))
    if _trace:
        return ret, res
    return ret
